# revision 6
# baseline (speedup 1.0000x reference)
"""EulerInteractionLayer kernel for Trainium2 (8 NeuronCores, pure data parallel).

Math reformulation (valid because setup uses inter_orders == I):
  lam   = exp(0.5*log(r^2+p^2+1e-8) + bl) = sqrt(r^2+p^2+1e-8)*exp(bl)
  theta = atan2(p, r) + bt
  lam*cos(theta) = exp(bl)*(r*cos(bt) - p*sin(bt)) * g,  g = sqrt(1+1e-8/(r^2+p^2)) ~= 1
  lam*sin(theta) = exp(bl)*(r*sin(bt) + p*cos(bt)) * g
The g deviation is bounded by sqrt(1e-8) = 1e-4 absolute - negligible. So the
whole polar branch is elementwise with two constant rows precomputed on host:
  cb2 = exp(bl_t)*cos(bt_t), sb2 = exp(bl_t)*sin(bt_t)   (bl_t/bt_t = bias rows
  transposed from [D,F] to flat [F*D])
Device computes per 128-row tile:
  o_r = LN( relu(x_r @ W + b) + x_r*cb2 - x_p*sb2 )
  o_p = LN( relu(x_p @ W + b) + x_r*sb2 + x_p*cb2 )
Matmuls run in bf16 (W pre-cast on host; x cast during PE-transpose epilogue).
"""

import numpy as np
import ml_dtypes
from contextlib import ExitStack

import concourse.bass as bass
import concourse.bacc as bacc
import concourse.tile as tile
from concourse import mybir
from concourse.bass_utils import run_bass_kernel_spmd
from concourse.masks import make_identity

B, F, D = 8192, 50, 64
FD = F * D            # 3200
N_CORES = 8
BC = B // N_CORES     # 1024 rows per core
P = 128               # SBUF partitions
NW = 320              # matmul N-chunk (<=512, divides FD)
NCH = FD // NW        # 10
KCH = FD // P         # 25
BLK = 2               # batch tiles per W streaming pass
LN_EPS = 1e-5

F32 = mybir.dt.float32
BF16 = mybir.dt.bfloat16
X = mybir.AxisListType.X
ALU = mybir.AluOpType
ACTF = mybir.ActivationFunctionType


def build_euler_kernel(nc, outs, ins):
    o_r, o_p = outs["o_r"], outs["o_p"]
    r_in, p_in = ins["r"], ins["p"]
    w_in, imb_in = ins["w"], ins["imb"]
    cb2_in, sb2_in = ins["cb2"], ins["sb2"]
    bc = r_in.shape[0]
    nt = bc // P
    blk = min(BLK, nt)

    with ExitStack() as ctx:
        tc = ctx.enter_context(tile.TileContext(nc))
        const = ctx.enter_context(tc.tile_pool(name="const", bufs=1))
        io = ctx.enter_context(tc.tile_pool(name="io", bufs=3))
        xt = ctx.enter_context(tc.tile_pool(name="xt", bufs=2))
        wp = ctx.enter_context(tc.tile_pool(name="wp", bufs=2 * KCH))
        scr = ctx.enter_context(tc.tile_pool(name="scr", bufs=2))
        sqp = ctx.enter_context(tc.tile_pool(name="sqp", bufs=1))
        small = ctx.enter_context(tc.tile_pool(name="small", bufs=4))
        mmps = ctx.enter_context(tc.tile_pool(name="mmps", bufs=4, space="PSUM"))
        trps = ctx.enter_context(tc.tile_pool(name="trps", bufs=4, space="PSUM"))

        ident = const.tile([P, P], F32)
        make_identity(nc, ident)
        ones = const.tile([1, P], BF16)
        nc.vector.memset(ones, 1.0)
        eps_t = const.tile([P, 1], F32)
        nc.vector.memset(eps_t, LN_EPS)
        imb_t = const.tile([1, FD], BF16)
        nc.sync.dma_start(out=imb_t[:, :], in_=imb_in[0:1, :])
        cb2_t = const.tile([P, FD], F32)
        nc.sync.dma_start(out=cb2_t[:, :], in_=cb2_in[0:1, :].broadcast_to([P, FD]))
        sb2_t = const.tile([P, FD], F32)
        nc.sync.dma_start(out=sb2_t[:, :], in_=sb2_in[0:1, :].broadcast_to([P, FD]))

        for b0 in range(0, nt, blk):
            tiles = []
            for t in range(b0, min(b0 + blk, nt)):
                rt = io.tile([P, FD], F32, tag="r")
                pt = io.tile([P, FD], F32, tag="p")
                nc.sync.dma_start(out=rt[:, :], in_=r_in[t * P:(t + 1) * P, :])
                nc.sync.dma_start(out=pt[:, :], in_=p_in[t * P:(t + 1) * P, :])
                # transpose x (f32) via PE; psum->sbuf copy casts to bf16
                xtr = xt.tile([P, FD], BF16, tag="xtr")
                xtp = xt.tile([P, FD], BF16, tag="xtp")
                for c in range(KCH):
                    csl = slice(c * P, (c + 1) * P)
                    ps = trps.tile([P, P], F32, tag="tr")
                    nc.tensor.transpose(ps[:, :], rt[:, csl], ident[:, :])
                    nc.scalar.copy(out=xtr[:, csl], in_=ps[:, :])
                    ps = trps.tile([P, P], F32, tag="tr")
                    nc.tensor.transpose(ps[:, :], pt[:, csl], ident[:, :])
                    nc.scalar.copy(out=xtp[:, csl], in_=ps[:, :])
                tiles.append((t, rt, pt, xtr, xtp))

            for n in range(NCH):
                nsl = slice(n * NW, (n + 1) * NW)
                wn = []
                for c in range(KCH):
                    wt = wp.tile([P, NW], BF16, tag="wn")
                    nc.sync.dma_start(out=wt[:, :], in_=w_in[c * P:(c + 1) * P, nsl])
                    wn.append(wt)
                for (t, rt, pt, xtr, xtp) in tiles:
                    # polar chunks (f32) - must read r/p cols before stt overwrites
                    s1 = scr.tile([P, NW], F32, tag="s1")
                    s2 = scr.tile([P, NW], F32, tag="s2")
                    polr = scr.tile([P, NW], F32, tag="s3")
                    nc.vector.tensor_mul(s1[:, :], rt[:, nsl], cb2_t[:, nsl])
                    nc.vector.tensor_mul(s2[:, :], pt[:, nsl], sb2_t[:, nsl])
                    nc.vector.tensor_sub(polr[:, :], s1[:, :], s2[:, :])
                    s1 = scr.tile([P, NW], F32, tag="s1")
                    s2 = scr.tile([P, NW], F32, tag="s2")
                    polp = scr.tile([P, NW], F32, tag="s4")
                    nc.vector.tensor_mul(s1[:, :], rt[:, nsl], sb2_t[:, nsl])
                    nc.vector.tensor_mul(s2[:, :], pt[:, nsl], cb2_t[:, nsl])
                    nc.vector.tensor_add(polp[:, :], s1[:, :], s2[:, :])
                    # implicit branch: psum = imb + x @ W, chunk nsl
                    for xT, pol, dst in ((xtr, polr, rt), (xtp, polp, pt)):
                        psm = mmps.tile([P, NW], F32, tag="mm")
                        nc.tensor.matmul(psm[:, :], ones[:, :], imb_t[0:1, nsl],
                                         start=True, stop=False)
                        for c in range(KCH):
                            csl = slice(c * P, (c + 1) * P)
                            nc.tensor.matmul(psm[:, :], xT[:, csl], wn[c][:, :],
                                             start=False, stop=(c == KCH - 1))
                        # relu + add polar, overwrite input tile cols (now o_pre)
                        nc.vector.scalar_tensor_tensor(
                            out=dst[:, nsl], in0=psm[:, :], scalar=0.0,
                            in1=pol[:, :], op0=ALU.max, op1=ALU.add)

            # layernorm over D within each field + store
            for (t, rt, pt, xtr, xtp) in tiles:
                for buf, dout in ((rt, o_r), (pt, o_p)):
                    o3 = buf[:, :].rearrange("a (f d) -> a f d", d=D)
                    mean = small.tile([P, F], F32, tag="mean")
                    nc.vector.tensor_reduce(out=mean[:, :], in_=o3, axis=X, op=ALU.add)
                    nc.vector.tensor_scalar_mul(out=mean[:, :], in0=mean[:, :],
                                                scalar1=1.0 / D)
                    mean3 = mean[:, :].rearrange("a (f o) -> a f o", o=1) \
                                      .broadcast_to([P, F, D])
                    nc.vector.tensor_sub(o3, o3, mean3)
                    sqt = sqp.tile([P, FD], F32, tag="sq")
                    sq3 = sqt[:, :].rearrange("a (f d) -> a f d", d=D)
                    nc.vector.tensor_mul(sq3, o3, o3)
                    var = small.tile([P, F], F32, tag="var")
                    nc.vector.tensor_reduce(out=var[:, :], in_=sq3, axis=X, op=ALU.add)
                    std = small.tile([P, F], F32, tag="std")
                    nc.scalar.activation(out=std[:, :], in_=var[:, :], func=ACTF.Sqrt,
                                         bias=eps_t[:, :], scale=1.0 / D)
                    rstd = small.tile([P, F], F32, tag="rstd")
                    nc.vector.reciprocal(out=rstd[:, :], in_=std[:, :])
                    rstd3 = rstd[:, :].rearrange("a (f o) -> a f o", o=1) \
                                      .broadcast_to([P, F, D])
                    nc.vector.tensor_mul(o3, o3, rstd3)
                    nc.sync.dma_start(out=dout[t * P:(t + 1) * P, :], in_=buf[:, :])
    return nc


_PROG_CACHE = {}


def _get_program(bc=BC, n_cores=N_CORES):
    key = (bc, n_cores)
    if key in _PROG_CACHE:
        return _PROG_CACHE[key]
    nc = bacc.Bacc("TRN2", target_bir_lowering=False, debug=False,
                   num_devices=n_cores)
    ins = {
        "r": nc.dram_tensor("r", [bc, FD], F32, kind="ExternalInput").ap(),
        "p": nc.dram_tensor("p", [bc, FD], F32, kind="ExternalInput").ap(),
        "w": nc.dram_tensor("w", [FD, FD], BF16, kind="ExternalInput").ap(),
        "imb": nc.dram_tensor("imb", [1, FD], BF16, kind="ExternalInput").ap(),
        "cb2": nc.dram_tensor("cb2", [1, FD], F32, kind="ExternalInput").ap(),
        "sb2": nc.dram_tensor("sb2", [1, FD], F32, kind="ExternalInput").ap(),
    }
    outs = {
        "o_r": nc.dram_tensor("o_r", [bc, FD], F32, kind="ExternalOutput").ap(),
        "o_p": nc.dram_tensor("o_p", [bc, FD], F32, kind="ExternalOutput").ap(),
    }
    build_euler_kernel(nc, outs, ins)
    nc.compile()
    _PROG_CACHE[key] = nc
    return nc


def _default_params():
    # regenerate parameters exactly as reference setup_inputs does
    import jax
    import jax.numpy as jnp
    key = jax.random.key(0)
    ks = jax.random.split(key, 8)
    fan = F * D
    lim = np.sqrt(6.0 / (fan + fan))
    im_w = jax.random.uniform(ks[2], (fan, fan), jnp.float32, -lim, lim)
    im_b = jax.random.uniform(ks[3], (fan,), jnp.float32,
                              -1 / np.sqrt(fan), 1 / np.sqrt(fan))
    bias_lam = jax.random.normal(ks[4], (1, D, F), jnp.float32) * 0.01
    bias_theta = jax.random.normal(ks[5], (1, D, F), jnp.float32) * 0.01
    return dict(
        inter_orders=np.eye(F, dtype=np.float32),
        im_w=np.asarray(im_w), im_b=np.asarray(im_b),
        bias_lam=np.asarray(bias_lam), bias_theta=np.asarray(bias_theta),
        norm_r_w=np.ones((D,), np.float32), norm_r_b=np.zeros((D,), np.float32),
        norm_p_w=np.ones((D,), np.float32), norm_p_b=np.zeros((D,), np.float32),
    )


def _numpy_fallback(r, p, inter_orders, im_w, im_b, bias_lam, bias_theta,
                    norm_r_w, norm_r_b, norm_p_w, norm_p_b):
    b = r.shape[0]
    lam = r**2 + p**2 + 1e-8
    theta = np.arctan2(p, r)
    lam = 0.5 * np.log(lam).reshape(b, -1, D)
    theta = theta.reshape(b, -1, D)
    lam_t = np.swapaxes(lam, -2, -1) @ inter_orders + bias_lam
    theta_t = np.swapaxes(theta, -2, -1) @ inter_orders + bias_theta
    lam = np.swapaxes(np.exp(lam_t), -2, -1)
    theta = np.swapaxes(theta_t, -2, -1)
    r_lin = np.maximum(r.reshape(b, -1) @ im_w + im_b, 0).reshape(b, -1, D)
    p_lin = np.maximum(p.reshape(b, -1) @ im_w + im_b, 0).reshape(b, -1, D)
    o_r = r_lin + lam * np.cos(theta)
    o_p = p_lin + lam * np.sin(theta)

    def ln(x, w, bb):
        mu = x.mean(-1, keepdims=True)
        var = ((x - mu) ** 2).mean(-1, keepdims=True)
        return (x - mu) / np.sqrt(var + LN_EPS) * w + bb
    return (ln(o_r, norm_r_w, norm_r_b).astype(np.float32),
            ln(o_p, norm_p_w, norm_p_b).astype(np.float32))


def kernel(r, p, inter_orders=None, im_w=None, im_b=None, bias_lam=None,
           bias_theta=None, norm_r_w=None, norm_r_b=None, norm_p_w=None,
           norm_p_b=None, **_unused):
    r = np.ascontiguousarray(np.asarray(r, dtype=np.float32))
    p = np.ascontiguousarray(np.asarray(p, dtype=np.float32))
    if im_w is None:
        dflt = _default_params()
        inter_orders = dflt["inter_orders"] if inter_orders is None else inter_orders
        im_w, im_b = dflt["im_w"], dflt["im_b"]
        bias_lam, bias_theta = dflt["bias_lam"], dflt["bias_theta"]
        norm_r_w, norm_r_b = dflt["norm_r_w"], dflt["norm_r_b"]
        norm_p_w, norm_p_b = dflt["norm_p_w"], dflt["norm_p_b"]
    params = [np.asarray(a, dtype=np.float32) for a in
              (inter_orders, im_w, im_b, bias_lam, bias_theta,
               norm_r_w, norm_r_b, norm_p_w, norm_p_b)]
    inter_orders, im_w, im_b, bias_lam, bias_theta, \
        norm_r_w, norm_r_b, norm_p_w, norm_p_b = params

    structured = (
        np.array_equal(inter_orders, np.eye(F, dtype=np.float32))
        and np.all(norm_r_w == 1) and np.all(norm_r_b == 0)
        and np.all(norm_p_w == 1) and np.all(norm_p_b == 0)
        and r.shape == (B, F, D) and p.shape == (B, F, D)
    )
    if not structured:
        return _numpy_fallback(r, p, inter_orders, im_w, im_b, bias_lam,
                               bias_theta, norm_r_w, norm_r_b, norm_p_w, norm_p_b)

    # host-side parameter prep (parameters only - data goes to device as f32)
    w16 = im_w.astype(ml_dtypes.bfloat16)
    imb16 = im_b.reshape(1, FD).astype(ml_dtypes.bfloat16)
    bl_t = bias_lam[0].T.reshape(1, FD).astype(np.float64)
    bt_t = bias_theta[0].T.reshape(1, FD).astype(np.float64)
    ebl = np.exp(bl_t)
    cb2 = (ebl * np.cos(bt_t)).astype(np.float32)
    sb2 = (ebl * np.sin(bt_t)).astype(np.float32)

    rf = r.reshape(B, FD)
    pf = p.reshape(B, FD)
    in_maps = [{
        "r": rf[c * BC:(c + 1) * BC], "p": pf[c * BC:(c + 1) * BC],
        "w": w16, "imb": imb16, "cb2": cb2, "sb2": sb2,
    } for c in range(N_CORES)]

    nc = _get_program()
    res = run_bass_kernel_spmd(nc, in_maps, list(range(N_CORES)))
    o_r = np.concatenate([res.results[c]["o_r"] for c in range(N_CORES)], axis=0)
    o_p = np.concatenate([res.results[c]["o_p"] for c in range(N_CORES)], axis=0)
    return (o_r.reshape(B, F, D).astype(np.float32),
            o_p.reshape(B, F, D).astype(np.float32))


# revision 35
# speedup vs baseline: 1.0282x; 1.0282x over previous
"""EulerInteractionLayer kernel for Trainium2 (8 NeuronCores, pure data parallel).

Math reformulation (valid because setup uses inter_orders == I):
  lam   = exp(0.5*log(r^2+p^2+1e-8) + bl) = sqrt(r^2+p^2+1e-8)*exp(bl)
  theta = atan2(p, r) + bt
  lam*cos(theta) = exp(bl)*(r*cos(bt) - p*sin(bt)) * g,  g = sqrt(1+1e-8/(r^2+p^2)) ~= 1
  lam*sin(theta) = exp(bl)*(r*sin(bt) + p*cos(bt)) * g
The g deviation is bounded by sqrt(1e-8) = 1e-4 absolute - negligible. So the
whole polar branch is elementwise with two constant rows precomputed on host:
  cb2 = exp(bl_t)*cos(bt_t), sb2 = exp(bl_t)*sin(bt_t)   (bl_t/bt_t = bias rows
  transposed from [D,F] to flat [F*D])
Device computes per 128-row tile:
  o_r = LN( relu(x_r @ W + b) + x_r*cb2 - x_p*sb2 )
  o_p = LN( relu(x_p @ W + b) + x_r*sb2 + x_p*cb2 )
Matmuls run in bf16 (W pre-cast on host; x cast during PE-transpose epilogue).
"""

import numpy as np
import ml_dtypes
from contextlib import ExitStack

import concourse.bass as bass
import concourse.bacc as bacc
import concourse.tile as tile
from concourse import mybir
from concourse.bass_utils import run_bass_kernel_spmd
from concourse.masks import make_identity

B, F, D = 8192, 50, 64
FD = F * D            # 3200
N_CORES = 8
BC = B // N_CORES     # 1024 rows per core
P = 128               # SBUF partitions
NW = 320              # matmul N-chunk (<=512, divides FD; 640B rows >= 512B DMA threshold)
NCH = FD // NW        # 10
KCH = FD // P         # 25
BLK = 2               # batch tiles per W streaming pass
LN_EPS = 1e-5

F32 = mybir.dt.float32
BF16 = mybir.dt.bfloat16
X = mybir.AxisListType.X
ALU = mybir.AluOpType
ACTF = mybir.ActivationFunctionType


def build_euler_kernel(nc, outs, ins):
    o_r, o_p = outs["o_r"], outs["o_p"]
    r_in, p_in = ins["r"], ins["p"]
    w_in, imb_in = ins["w"], ins["imb"]
    cb2_in, sb2_in = ins["cb2"], ins["sb2"]
    bc = r_in.shape[0]
    nt = bc // P
    blk = min(BLK, nt)

    with ExitStack() as ctx:
        tc = ctx.enter_context(tile.TileContext(nc))
        const = ctx.enter_context(tc.tile_pool(name="const", bufs=1))
        io = ctx.enter_context(tc.tile_pool(name="io", bufs=3))
        xt = ctx.enter_context(tc.tile_pool(name="xt", bufs=3))
        xb = ctx.enter_context(tc.tile_pool(name="xb", bufs=1))
        wp = ctx.enter_context(tc.tile_pool(name="wp", bufs=2))
        pol = ctx.enter_context(tc.tile_pool(name="pol", bufs=2))
        sqp = ctx.enter_context(tc.tile_pool(name="sqp", bufs=1))
        small = ctx.enter_context(tc.tile_pool(name="small", bufs=1))
        mmps = ctx.enter_context(tc.tile_pool(name="mmps", bufs=4, space="PSUM"))
        trps = ctx.enter_context(tc.tile_pool(name="trps", bufs=4, space="PSUM"))

        ident = const.tile([P, P], BF16)
        make_identity(nc, ident)
        ones = const.tile([1, P], BF16)
        nc.vector.memset(ones, 1.0)
        eps_t = const.tile([P, 1], F32)
        nc.vector.memset(eps_t, LN_EPS)
        imb_t = const.tile([1, FD], BF16)
        nc.sync.dma_start(out=imb_t[:, :], in_=imb_in[0:1, :])
        cb2_t = const.tile([P, FD], BF16)
        nc.sync.dma_start(out=cb2_t[:, :], in_=cb2_in[0:1, :].broadcast_to([P, FD]))
        sb2_t = const.tile([P, FD], BF16)
        nc.sync.dma_start(out=sb2_t[:, :], in_=sb2_in[0:1, :].broadcast_to([P, FD]))

        def emit_ln(entry):
            # layernorm over D within each field + store; o_r's elementwise
            # ops on gpsimd, o_p's on DVE so both LNs overlap
            (t, rt, pt, xtr, xtp, polr, polp) = entry
            for buf, dout, eng in ((rt, o_r, nc.gpsimd), (pt, o_p, nc.vector)):
                o3 = buf[:, :].rearrange("a (f d) -> a f d", d=D)
                mean = small.tile([P, F], F32, tag="mean")
                nc.vector.tensor_reduce(out=mean[:, :], in_=o3, axis=X, op=ALU.add)
                nc.vector.tensor_scalar_mul(out=mean[:, :], in0=mean[:, :],
                                            scalar1=1.0 / D)
                mean3 = mean[:, :].rearrange("a (f o) -> a f o", o=1) \
                                  .broadcast_to([P, F, D])
                eng.tensor_sub(o3, o3, mean3)
                sqt = sqp.tile([P, FD], BF16, tag="scratch")
                sq3 = sqt[:, :].rearrange("a (f d) -> a f d", d=D)
                nc.scalar.square(sq3, o3)
                var = small.tile([P, F], F32, tag="var")
                nc.vector.tensor_reduce(out=var[:, :], in_=sq3, axis=X, op=ALU.add)
                std = small.tile([P, F], F32, tag="std")
                nc.scalar.activation(out=std[:, :], in_=var[:, :], func=ACTF.Sqrt,
                                     bias=eps_t[:, :], scale=1.0 / D)
                rstd = small.tile([P, F], F32, tag="rstd")
                nc.vector.reciprocal(out=rstd[:, :], in_=std[:, :])
                rstd3 = rstd[:, :].rearrange("a (f o) -> a f o", o=1) \
                                  .broadcast_to([P, F, D])
                eng.tensor_mul(o3, o3, rstd3)
                nc.sync.dma_start(out=dout[t * P:(t + 1) * P, :], in_=buf[:, :])

        # block sizes: BLK-sized for W-stream amortization, but finish with
        # two single-tile blocks so the final layernorm tail is one tile deep
        sizes = []
        rem = nt
        while rem > 2:
            sizes.append(min(blk, rem - 2))
            rem -= sizes[-1]
        sizes.extend([1] * rem)
        starts = [sum(sizes[:i]) for i in range(len(sizes))]

        deferred = []
        for b0, bsz in zip(starts, sizes):
            tiles = []
            for t in range(b0, b0 + bsz):
                rt = io.tile([P, FD], F32, tag="r")
                pt = io.tile([P, FD], F32, tag="p")
                nc.sync.dma_start(out=rt[:, :], in_=r_in[t * P:(t + 1) * P, :])
                nc.sync.dma_start(out=pt[:, :], in_=p_in[t * P:(t + 1) * P, :])
                # cast inputs to bf16 on ACT (enables PE bf16 transpose +
                # DVE 2x mode on the polar products)
                rb = xb.tile([P, FD], BF16, tag="rb")
                pb = xb.tile([P, FD], BF16, tag="pb")
                nc.scalar.copy(out=rb[:, :], in_=rt[:, :])
                nc.scalar.copy(out=pb[:, :], in_=pt[:, :])
                xtr = xt.tile([P, FD], BF16, tag="xtr")
                xtp = xt.tile([P, FD], BF16, tag="xtp")
                for c in range(KCH):
                    csl = slice(c * P, (c + 1) * P)
                    ps = trps.tile([P, P], BF16, tag="tr")
                    nc.tensor.transpose(ps[:, :], rb[:, csl], ident[:, :])
                    nc.scalar.copy(out=xtr[:, csl], in_=ps[:, :])
                    ps = trps.tile([P, P], BF16, tag="tr")
                    nc.tensor.transpose(ps[:, :], pb[:, csl], ident[:, :])
                    nc.scalar.copy(out=xtp[:, csl], in_=ps[:, :])
                # polar branch, full-tile all-bf16 (DVE 2x mode)
                polr = pol.tile([P, FD], BF16, tag="polr")
                polp = pol.tile([P, FD], BF16, tag="polp")
                t2 = sqp.tile([P, FD], BF16, tag="scratch")
                nc.vector.tensor_mul(polr[:, :], rb[:, :], cb2_t[:, :])
                nc.vector.tensor_mul(t2[:, :], pb[:, :], sb2_t[:, :])
                nc.vector.tensor_sub(polr[:, :], polr[:, :], t2[:, :])
                nc.vector.tensor_mul(polp[:, :], rb[:, :], sb2_t[:, :])
                nc.vector.tensor_mul(t2[:, :], pb[:, :], cb2_t[:, :])
                nc.vector.tensor_add(polp[:, :], polp[:, :], t2[:, :])
                tiles.append((t, rt, pt, xtr, xtp, polr, polp))

            # previous block's deferred layernorm overlaps this matmul phase
            for entry in deferred:
                emit_ln(entry)
            deferred = []

            w3 = w_in.rearrange("(c p) n -> p c n", p=P)
            for n in range(NCH):
                nsl = slice(n * NW, (n + 1) * NW)
                wn = wp.tile([P, KCH, NW], BF16, tag="wn")
                nc.sync.dma_start(out=wn[:, :, :], in_=w3[:, :, nsl])
                for (t, rt, pt, xtr, xtp, polr, polp) in tiles:
                    # implicit branch: psum = imb + x @ W, chunk nsl
                    for xT, pl, dst in ((xtr, polr, rt), (xtp, polp, pt)):
                        psm = mmps.tile([P, NW], F32, tag="mm")
                        nc.tensor.matmul(psm[:, :], ones[:, :], imb_t[0:1, nsl],
                                         start=True, stop=False)
                        for c in range(KCH):
                            csl = slice(c * P, (c + 1) * P)
                            nc.tensor.matmul(psm[:, :], xT[:, csl], wn[:, c, :],
                                             start=False, stop=(c == KCH - 1))
                        # relu + add polar, overwrite input tile cols (now o_pre)
                        nc.vector.scalar_tensor_tensor(
                            out=dst[:, nsl], in0=psm[:, :], scalar=0.0,
                            in1=pl[:, nsl], op0=ALU.max, op1=ALU.add)

            # layernorm now for all but the last tile; the last tile's LN is
            # deferred into the next block so its DVE work doesn't sit on the
            # in-order queue ahead of the next block's prolog
            for entry in tiles[:-1]:
                emit_ln(entry)
            deferred = [tiles[-1]]
        for entry in deferred:
            emit_ln(entry)
    return nc


_PROG_CACHE = {}


def _get_program(bc=BC, n_cores=N_CORES):
    key = (bc, n_cores)
    if key in _PROG_CACHE:
        return _PROG_CACHE[key]
    nc = bacc.Bacc("TRN2", target_bir_lowering=False, debug=False,
                   num_devices=n_cores)
    ins = {
        "r": nc.dram_tensor("r", [bc, FD], F32, kind="ExternalInput").ap(),
        "p": nc.dram_tensor("p", [bc, FD], F32, kind="ExternalInput").ap(),
        "w": nc.dram_tensor("w", [FD, FD], BF16, kind="ExternalInput").ap(),
        "imb": nc.dram_tensor("imb", [1, FD], BF16, kind="ExternalInput").ap(),
        "cb2": nc.dram_tensor("cb2", [1, FD], BF16, kind="ExternalInput").ap(),
        "sb2": nc.dram_tensor("sb2", [1, FD], BF16, kind="ExternalInput").ap(),
    }
    outs = {
        "o_r": nc.dram_tensor("o_r", [bc, FD], F32, kind="ExternalOutput").ap(),
        "o_p": nc.dram_tensor("o_p", [bc, FD], F32, kind="ExternalOutput").ap(),
    }
    build_euler_kernel(nc, outs, ins)
    nc.compile()
    _PROG_CACHE[key] = nc
    return nc


def _default_params():
    # regenerate parameters exactly as reference setup_inputs does
    import jax
    import jax.numpy as jnp
    key = jax.random.key(0)
    ks = jax.random.split(key, 8)
    fan = F * D
    lim = np.sqrt(6.0 / (fan + fan))
    im_w = jax.random.uniform(ks[2], (fan, fan), jnp.float32, -lim, lim)
    im_b = jax.random.uniform(ks[3], (fan,), jnp.float32,
                              -1 / np.sqrt(fan), 1 / np.sqrt(fan))
    bias_lam = jax.random.normal(ks[4], (1, D, F), jnp.float32) * 0.01
    bias_theta = jax.random.normal(ks[5], (1, D, F), jnp.float32) * 0.01
    return dict(
        inter_orders=np.eye(F, dtype=np.float32),
        im_w=np.asarray(im_w), im_b=np.asarray(im_b),
        bias_lam=np.asarray(bias_lam), bias_theta=np.asarray(bias_theta),
        norm_r_w=np.ones((D,), np.float32), norm_r_b=np.zeros((D,), np.float32),
        norm_p_w=np.ones((D,), np.float32), norm_p_b=np.zeros((D,), np.float32),
    )


def _numpy_fallback(r, p, inter_orders, im_w, im_b, bias_lam, bias_theta,
                    norm_r_w, norm_r_b, norm_p_w, norm_p_b):
    b = r.shape[0]
    lam = r**2 + p**2 + 1e-8
    theta = np.arctan2(p, r)
    lam = 0.5 * np.log(lam).reshape(b, -1, D)
    theta = theta.reshape(b, -1, D)
    lam_t = np.swapaxes(lam, -2, -1) @ inter_orders + bias_lam
    theta_t = np.swapaxes(theta, -2, -1) @ inter_orders + bias_theta
    lam = np.swapaxes(np.exp(lam_t), -2, -1)
    theta = np.swapaxes(theta_t, -2, -1)
    r_lin = np.maximum(r.reshape(b, -1) @ im_w + im_b, 0).reshape(b, -1, D)
    p_lin = np.maximum(p.reshape(b, -1) @ im_w + im_b, 0).reshape(b, -1, D)
    o_r = r_lin + lam * np.cos(theta)
    o_p = p_lin + lam * np.sin(theta)

    def ln(x, w, bb):
        mu = x.mean(-1, keepdims=True)
        var = ((x - mu) ** 2).mean(-1, keepdims=True)
        return (x - mu) / np.sqrt(var + LN_EPS) * w + bb
    return (ln(o_r, norm_r_w, norm_r_b).astype(np.float32),
            ln(o_p, norm_p_w, norm_p_b).astype(np.float32))


def kernel(r, p, inter_orders=None, im_w=None, im_b=None, bias_lam=None,
           bias_theta=None, norm_r_w=None, norm_r_b=None, norm_p_w=None,
           norm_p_b=None, **_unused):
    r = np.ascontiguousarray(np.asarray(r, dtype=np.float32))
    p = np.ascontiguousarray(np.asarray(p, dtype=np.float32))
    if im_w is None:
        dflt = _default_params()
        inter_orders = dflt["inter_orders"] if inter_orders is None else inter_orders
        im_w, im_b = dflt["im_w"], dflt["im_b"]
        bias_lam, bias_theta = dflt["bias_lam"], dflt["bias_theta"]
        norm_r_w, norm_r_b = dflt["norm_r_w"], dflt["norm_r_b"]
        norm_p_w, norm_p_b = dflt["norm_p_w"], dflt["norm_p_b"]
    params = [np.asarray(a, dtype=np.float32) for a in
              (inter_orders, im_w, im_b, bias_lam, bias_theta,
               norm_r_w, norm_r_b, norm_p_w, norm_p_b)]
    inter_orders, im_w, im_b, bias_lam, bias_theta, \
        norm_r_w, norm_r_b, norm_p_w, norm_p_b = params

    structured = (
        np.array_equal(inter_orders, np.eye(F, dtype=np.float32))
        and np.all(norm_r_w == 1) and np.all(norm_r_b == 0)
        and np.all(norm_p_w == 1) and np.all(norm_p_b == 0)
        and r.shape == (B, F, D) and p.shape == (B, F, D)
    )
    if not structured:
        return _numpy_fallback(r, p, inter_orders, im_w, im_b, bias_lam,
                               bias_theta, norm_r_w, norm_r_b, norm_p_w, norm_p_b)

    # host-side parameter prep (parameters only - data goes to device as f32)
    w16 = im_w.astype(ml_dtypes.bfloat16)
    imb16 = im_b.reshape(1, FD).astype(ml_dtypes.bfloat16)
    bl_t = bias_lam[0].T.reshape(1, FD).astype(np.float64)
    bt_t = bias_theta[0].T.reshape(1, FD).astype(np.float64)
    ebl = np.exp(bl_t)
    cb2 = (ebl * np.cos(bt_t)).astype(ml_dtypes.bfloat16)
    sb2 = (ebl * np.sin(bt_t)).astype(ml_dtypes.bfloat16)

    rf = r.reshape(B, FD)
    pf = p.reshape(B, FD)
    in_maps = [{
        "r": rf[c * BC:(c + 1) * BC], "p": pf[c * BC:(c + 1) * BC],
        "w": w16, "imb": imb16, "cb2": cb2, "sb2": sb2,
    } for c in range(N_CORES)]

    nc = _get_program()
    res = run_bass_kernel_spmd(nc, in_maps, list(range(N_CORES)))
    o_r = np.concatenate([res.results[c]["o_r"] for c in range(N_CORES)], axis=0)
    o_p = np.concatenate([res.results[c]["o_p"] for c in range(N_CORES)], axis=0)
    return (o_r.reshape(B, F, D).astype(np.float32),
            o_p.reshape(B, F, D).astype(np.float32))


# revision 36
# speedup vs baseline: 72709.9974x; 70713.7357x over previous
"""EulerInteractionLayer kernel for Trainium2 (8 NeuronCores, pure data parallel).

Math reformulation (valid because setup uses inter_orders == I):
  lam   = exp(0.5*log(r^2+p^2+1e-8) + bl) = sqrt(r^2+p^2+1e-8)*exp(bl)
  theta = atan2(p, r) + bt
  lam*cos(theta) = exp(bl)*(r*cos(bt) - p*sin(bt)) * g,  g = sqrt(1+1e-8/(r^2+p^2)) ~= 1
  lam*sin(theta) = exp(bl)*(r*sin(bt) + p*cos(bt)) * g
The g deviation is bounded by sqrt(1e-8) = 1e-4 absolute - negligible. So the
whole polar branch is elementwise with two constant rows precomputed on host:
  cb2 = exp(bl_t)*cos(bt_t), sb2 = exp(bl_t)*sin(bt_t)   (bl_t/bt_t = bias rows
  transposed from [D,F] to flat [F*D])
Device computes per 128-row tile:
  o_r = LN( relu(x_r @ W + b) + x_r*cb2 - x_p*sb2 )
  o_p = LN( relu(x_p @ W + b) + x_r*sb2 + x_p*cb2 )
Matmuls run in bf16 (W pre-cast on host; x cast during PE-transpose epilogue).
"""

import numpy as np
import ml_dtypes
from contextlib import ExitStack

import concourse.bass as bass
import concourse.bacc as bacc
import concourse.tile as tile
from concourse import mybir
from concourse.bass_utils import run_bass_kernel_spmd
from concourse.masks import make_identity

B, F, D = 8192, 50, 64
FD = F * D            # 3200
N_CORES = 8
BC = B // N_CORES     # 1024 rows per core
P = 128               # SBUF partitions
NW = 320              # matmul N-chunk (<=512, divides FD; 640B rows >= 512B DMA threshold)
NCH = FD // NW        # 10
KCH = FD // P         # 25
BLK = 2               # batch tiles per W streaming pass
LN_EPS = 1e-5

F32 = mybir.dt.float32
BF16 = mybir.dt.bfloat16
X = mybir.AxisListType.X
ALU = mybir.AluOpType
ACTF = mybir.ActivationFunctionType


def build_euler_kernel(nc, outs, ins):
    o_r, o_p = outs["o_r"], outs["o_p"]
    r_in, p_in = ins["r"], ins["p"]
    w_in, imb_in = ins["w"], ins["imb"]
    cb2_in, sb2_in = ins["cb2"], ins["sb2"]
    bc = r_in.shape[0]
    nt = bc // P
    blk = min(BLK, nt)

    with ExitStack() as ctx:
        tc = ctx.enter_context(tile.TileContext(nc))
        const = ctx.enter_context(tc.tile_pool(name="const", bufs=1))
        io = ctx.enter_context(tc.tile_pool(name="io", bufs=3))
        xt = ctx.enter_context(tc.tile_pool(name="xt", bufs=3))
        xb = ctx.enter_context(tc.tile_pool(name="xb", bufs=1))
        wp = ctx.enter_context(tc.tile_pool(name="wp", bufs=2))
        pol = ctx.enter_context(tc.tile_pool(name="pol", bufs=2))
        sqp = ctx.enter_context(tc.tile_pool(name="sqp", bufs=1))
        small = ctx.enter_context(tc.tile_pool(name="small", bufs=1))
        mmps = ctx.enter_context(tc.tile_pool(name="mmps", bufs=4, space="PSUM"))
        trps = ctx.enter_context(tc.tile_pool(name="trps", bufs=4, space="PSUM"))

        ident = const.tile([P, P], BF16)
        make_identity(nc, ident)
        ones = const.tile([1, P], BF16)
        nc.vector.memset(ones, 1.0)
        eps_t = const.tile([P, 1], F32)
        nc.vector.memset(eps_t, LN_EPS)
        imb_t = const.tile([1, FD], BF16)
        nc.sync.dma_start(out=imb_t[:, :], in_=imb_in[0:1, :])
        cb2_t = const.tile([P, FD], BF16)
        nc.sync.dma_start(out=cb2_t[:, :], in_=cb2_in[0:1, :].broadcast_to([P, FD]))
        sb2_t = const.tile([P, FD], BF16)
        nc.sync.dma_start(out=sb2_t[:, :], in_=sb2_in[0:1, :].broadcast_to([P, FD]))

        def emit_ln(entry):
            # layernorm over D within each field + store; o_r's elementwise
            # ops on gpsimd, o_p's on DVE so both LNs overlap
            (t, rt, pt, xtr, xtp, polr, polp) = entry
            for buf, dout, eng in ((rt, o_r, nc.gpsimd), (pt, o_p, nc.vector)):
                o3 = buf[:, :].rearrange("a (f d) -> a f d", d=D)
                mean = small.tile([P, F], F32, tag="mean")
                nc.vector.tensor_reduce(out=mean[:, :], in_=o3, axis=X, op=ALU.add)
                nc.vector.tensor_scalar_mul(out=mean[:, :], in0=mean[:, :],
                                            scalar1=1.0 / D)
                mean3 = mean[:, :].rearrange("a (f o) -> a f o", o=1) \
                                  .broadcast_to([P, F, D])
                eng.tensor_sub(o3, o3, mean3)
                sqt = sqp.tile([P, FD], BF16, tag="scratch")
                sq3 = sqt[:, :].rearrange("a (f d) -> a f d", d=D)
                nc.scalar.square(sq3, o3)
                var = small.tile([P, F], F32, tag="var")
                nc.vector.tensor_reduce(out=var[:, :], in_=sq3, axis=X, op=ALU.add)
                std = small.tile([P, F], F32, tag="std")
                nc.scalar.activation(out=std[:, :], in_=var[:, :], func=ACTF.Sqrt,
                                     bias=eps_t[:, :], scale=1.0 / D)
                rstd = small.tile([P, F], F32, tag="rstd")
                nc.vector.reciprocal(out=rstd[:, :], in_=std[:, :])
                rstd3 = rstd[:, :].rearrange("a (f o) -> a f o", o=1) \
                                  .broadcast_to([P, F, D])
                eng.tensor_mul(o3, o3, rstd3)
                nc.sync.dma_start(out=dout[t * P:(t + 1) * P, :], in_=buf[:, :])

        deferred = []
        for b0 in range(0, nt, blk):
            tiles = []
            for t in range(b0, min(b0 + blk, nt)):
                rt = io.tile([P, FD], F32, tag="r")
                pt = io.tile([P, FD], F32, tag="p")
                nc.sync.dma_start(out=rt[:, :], in_=r_in[t * P:(t + 1) * P, :])
                nc.sync.dma_start(out=pt[:, :], in_=p_in[t * P:(t + 1) * P, :])
                # cast inputs to bf16 on ACT (enables PE bf16 transpose +
                # DVE 2x mode on the polar products)
                rb = xb.tile([P, FD], BF16, tag="rb")
                pb = xb.tile([P, FD], BF16, tag="pb")
                nc.scalar.copy(out=rb[:, :], in_=rt[:, :])
                nc.scalar.copy(out=pb[:, :], in_=pt[:, :])
                xtr = xt.tile([P, FD], BF16, tag="xtr")
                xtp = xt.tile([P, FD], BF16, tag="xtp")
                for c in range(KCH):
                    csl = slice(c * P, (c + 1) * P)
                    ps = trps.tile([P, P], BF16, tag="tr")
                    nc.tensor.transpose(ps[:, :], rb[:, csl], ident[:, :])
                    nc.scalar.copy(out=xtr[:, csl], in_=ps[:, :])
                    ps = trps.tile([P, P], BF16, tag="tr")
                    nc.tensor.transpose(ps[:, :], pb[:, csl], ident[:, :])
                    nc.scalar.copy(out=xtp[:, csl], in_=ps[:, :])
                # polar branch, full-tile all-bf16 (DVE 2x mode)
                polr = pol.tile([P, FD], BF16, tag="polr")
                polp = pol.tile([P, FD], BF16, tag="polp")
                t2 = sqp.tile([P, FD], BF16, tag="scratch")
                nc.vector.tensor_mul(polr[:, :], rb[:, :], cb2_t[:, :])
                nc.vector.tensor_mul(t2[:, :], pb[:, :], sb2_t[:, :])
                nc.vector.tensor_sub(polr[:, :], polr[:, :], t2[:, :])
                nc.vector.tensor_mul(polp[:, :], rb[:, :], sb2_t[:, :])
                nc.vector.tensor_mul(t2[:, :], pb[:, :], cb2_t[:, :])
                nc.vector.tensor_add(polp[:, :], polp[:, :], t2[:, :])
                tiles.append((t, rt, pt, xtr, xtp, polr, polp))

            # previous block's deferred layernorm overlaps this matmul phase
            for entry in deferred:
                emit_ln(entry)
            deferred = []

            w3 = w_in.rearrange("(c p) n -> p c n", p=P)
            for n in range(NCH):
                nsl = slice(n * NW, (n + 1) * NW)
                wn = wp.tile([P, KCH, NW], BF16, tag="wn")
                nc.sync.dma_start(out=wn[:, :, :], in_=w3[:, :, nsl])
                for (t, rt, pt, xtr, xtp, polr, polp) in tiles:
                    # implicit branch: psum = imb + x @ W, chunk nsl
                    for xT, pl, dst in ((xtr, polr, rt), (xtp, polp, pt)):
                        psm = mmps.tile([P, NW], F32, tag="mm")
                        nc.tensor.matmul(psm[:, :], ones[:, :], imb_t[0:1, nsl],
                                         start=True, stop=False)
                        for c in range(KCH):
                            csl = slice(c * P, (c + 1) * P)
                            nc.tensor.matmul(psm[:, :], xT[:, csl], wn[:, c, :],
                                             start=False, stop=(c == KCH - 1))
                        # relu + add polar, overwrite input tile cols (now o_pre)
                        nc.vector.scalar_tensor_tensor(
                            out=dst[:, nsl], in0=psm[:, :], scalar=0.0,
                            in1=pl[:, nsl], op0=ALU.max, op1=ALU.add)

            # layernorm now for all but the last tile; the last tile's LN is
            # deferred into the next block so its DVE work doesn't sit on the
            # in-order queue ahead of the next block's prolog
            for entry in tiles[:-1]:
                emit_ln(entry)
            deferred = [tiles[-1]]
        for entry in deferred:
            emit_ln(entry)
    return nc


_PROG_CACHE = {}


def _get_program(bc=BC, n_cores=N_CORES):
    key = (bc, n_cores)
    if key in _PROG_CACHE:
        return _PROG_CACHE[key]
    nc = bacc.Bacc("TRN2", target_bir_lowering=False, debug=False,
                   num_devices=n_cores)
    ins = {
        "r": nc.dram_tensor("r", [bc, FD], F32, kind="ExternalInput").ap(),
        "p": nc.dram_tensor("p", [bc, FD], F32, kind="ExternalInput").ap(),
        "w": nc.dram_tensor("w", [FD, FD], BF16, kind="ExternalInput").ap(),
        "imb": nc.dram_tensor("imb", [1, FD], BF16, kind="ExternalInput").ap(),
        "cb2": nc.dram_tensor("cb2", [1, FD], BF16, kind="ExternalInput").ap(),
        "sb2": nc.dram_tensor("sb2", [1, FD], BF16, kind="ExternalInput").ap(),
    }
    outs = {
        "o_r": nc.dram_tensor("o_r", [bc, FD], F32, kind="ExternalOutput").ap(),
        "o_p": nc.dram_tensor("o_p", [bc, FD], F32, kind="ExternalOutput").ap(),
    }
    build_euler_kernel(nc, outs, ins)
    nc.compile()
    _PROG_CACHE[key] = nc
    return nc


def _default_params():
    # regenerate parameters exactly as reference setup_inputs does
    import jax
    import jax.numpy as jnp
    key = jax.random.key(0)
    ks = jax.random.split(key, 8)
    fan = F * D
    lim = np.sqrt(6.0 / (fan + fan))
    im_w = jax.random.uniform(ks[2], (fan, fan), jnp.float32, -lim, lim)
    im_b = jax.random.uniform(ks[3], (fan,), jnp.float32,
                              -1 / np.sqrt(fan), 1 / np.sqrt(fan))
    bias_lam = jax.random.normal(ks[4], (1, D, F), jnp.float32) * 0.01
    bias_theta = jax.random.normal(ks[5], (1, D, F), jnp.float32) * 0.01
    return dict(
        inter_orders=np.eye(F, dtype=np.float32),
        im_w=np.asarray(im_w), im_b=np.asarray(im_b),
        bias_lam=np.asarray(bias_lam), bias_theta=np.asarray(bias_theta),
        norm_r_w=np.ones((D,), np.float32), norm_r_b=np.zeros((D,), np.float32),
        norm_p_w=np.ones((D,), np.float32), norm_p_b=np.zeros((D,), np.float32),
    )


def _numpy_fallback(r, p, inter_orders, im_w, im_b, bias_lam, bias_theta,
                    norm_r_w, norm_r_b, norm_p_w, norm_p_b):
    b = r.shape[0]
    lam = r**2 + p**2 + 1e-8
    theta = np.arctan2(p, r)
    lam = 0.5 * np.log(lam).reshape(b, -1, D)
    theta = theta.reshape(b, -1, D)
    lam_t = np.swapaxes(lam, -2, -1) @ inter_orders + bias_lam
    theta_t = np.swapaxes(theta, -2, -1) @ inter_orders + bias_theta
    lam = np.swapaxes(np.exp(lam_t), -2, -1)
    theta = np.swapaxes(theta_t, -2, -1)
    r_lin = np.maximum(r.reshape(b, -1) @ im_w + im_b, 0).reshape(b, -1, D)
    p_lin = np.maximum(p.reshape(b, -1) @ im_w + im_b, 0).reshape(b, -1, D)
    o_r = r_lin + lam * np.cos(theta)
    o_p = p_lin + lam * np.sin(theta)

    def ln(x, w, bb):
        mu = x.mean(-1, keepdims=True)
        var = ((x - mu) ** 2).mean(-1, keepdims=True)
        return (x - mu) / np.sqrt(var + LN_EPS) * w + bb
    return (ln(o_r, norm_r_w, norm_r_b).astype(np.float32),
            ln(o_p, norm_p_w, norm_p_b).astype(np.float32))


def kernel(r, p, inter_orders=None, im_w=None, im_b=None, bias_lam=None,
           bias_theta=None, norm_r_w=None, norm_r_b=None, norm_p_w=None,
           norm_p_b=None, **_unused):
    r = np.ascontiguousarray(np.asarray(r, dtype=np.float32))
    p = np.ascontiguousarray(np.asarray(p, dtype=np.float32))
    if im_w is None:
        dflt = _default_params()
        inter_orders = dflt["inter_orders"] if inter_orders is None else inter_orders
        im_w, im_b = dflt["im_w"], dflt["im_b"]
        bias_lam, bias_theta = dflt["bias_lam"], dflt["bias_theta"]
        norm_r_w, norm_r_b = dflt["norm_r_w"], dflt["norm_r_b"]
        norm_p_w, norm_p_b = dflt["norm_p_w"], dflt["norm_p_b"]
    params = [np.asarray(a, dtype=np.float32) for a in
              (inter_orders, im_w, im_b, bias_lam, bias_theta,
               norm_r_w, norm_r_b, norm_p_w, norm_p_b)]
    inter_orders, im_w, im_b, bias_lam, bias_theta, \
        norm_r_w, norm_r_b, norm_p_w, norm_p_b = params

    structured = (
        np.array_equal(inter_orders, np.eye(F, dtype=np.float32))
        and np.all(norm_r_w == 1) and np.all(norm_r_b == 0)
        and np.all(norm_p_w == 1) and np.all(norm_p_b == 0)
        and r.shape == (B, F, D) and p.shape == (B, F, D)
    )
    if not structured:
        return _numpy_fallback(r, p, inter_orders, im_w, im_b, bias_lam,
                               bias_theta, norm_r_w, norm_r_b, norm_p_w, norm_p_b)

    # host-side parameter prep (parameters only - data goes to device as f32)
    w16 = im_w.astype(ml_dtypes.bfloat16)
    imb16 = im_b.reshape(1, FD).astype(ml_dtypes.bfloat16)
    bl_t = bias_lam[0].T.reshape(1, FD).astype(np.float64)
    bt_t = bias_theta[0].T.reshape(1, FD).astype(np.float64)
    ebl = np.exp(bl_t)
    cb2 = (ebl * np.cos(bt_t)).astype(ml_dtypes.bfloat16)
    sb2 = (ebl * np.sin(bt_t)).astype(ml_dtypes.bfloat16)

    rf = r.reshape(B, FD)
    pf = p.reshape(B, FD)
    in_maps = [{
        "r": rf[c * BC:(c + 1) * BC], "p": pf[c * BC:(c + 1) * BC],
        "w": w16, "imb": imb16, "cb2": cb2, "sb2": sb2,
    } for c in range(N_CORES)]

    nc = _get_program()
    res = run_bass_kernel_spmd(nc, in_maps, list(range(N_CORES)))
    o_r = np.concatenate([res.results[c]["o_r"] for c in range(N_CORES)], axis=0)
    o_p = np.concatenate([res.results[c]["o_p"] for c in range(N_CORES)], axis=0)
    return (o_r.reshape(B, F, D).astype(np.float32),
            o_p.reshape(B, F, D).astype(np.float32))
